# revision 2
# baseline (speedup 1.0000x reference)
"""LGA3 (3x local guided aggregation, radius 2) on 8 TRN2 NeuronCores.

Sharding: H split 8 ways (48 output rows/core), redundant-halo compute
(pass1 computes 56 rows, pass2 52, pass3 48) -> no inter-core comms.

Layout: partition = x (128-chunks), free = (row, d). Host pre-transposes
to [b, y, x, d] with x padded +-2 (772) and d padded +-1 (66), so every
spatial/disparity shift is a free-dim AP offset; the 5 x-shifts of one
row collapse into a contiguous 330-element window (x-stride == d-extent).
Weights go to [b, y, x, 75] with zeroed out-of-image rows (zero weights
reproduce the reference's zero-padding semantics for halo rows).

Compute: per tap t=(g,i,j): acc += cost_window * w[:, :, t] (broadcast
along d via a step-0 free dim) -- two fp32 tensor_tensor DVE ops per tap.
"""

import os
import sys

for _p in ("/opt/trn_rl_repo", "/root/.axon_site/_ro/trn_rl_repo"):
    if os.path.isdir(_p) and _p not in sys.path:
        sys.path.append(_p)

import numpy as np
import concourse.bass as bass
import concourse.mybir as mybir
from concourse.tile import TileContext
from concourse import bass_utils

F32 = mybir.dt.float32

B, D, H, W = 2, 64, 384, 768
N_CORES = 8
ROWS = H // N_CORES  # 48 output rows per core

# slab [b, 60 rows, 772 x, 66 d]  (strides in elements)
S_ROW, S_X, S_B = 772 * 66, 66, 60 * 772 * 66
SLAB_SHAPE = [2, 60, 772, 66]
# weight slab [b, 56 rows, 768 x, 75 t]
W_ROW, W_X, W_B = 768 * 75, 75, 56 * 768 * 75
W_SHAPE = [2, 56, 768, 75]
# output [b, 48 rows, 768 x, 64 d]
O_ROW, O_X, O_B = 768 * 64, 64, 48 * 768 * 64
O_SHAPE = [2, 48, 768, 64]

LAST_EXEC_NS = [None]


def _split_waits(nc, max_waits=1):
    """Split >max_waits sync waits on one instruction into preceding
    wait-only drains (walrus setupSyncWait limit workaround)."""
    ctr = [0]
    for f in nc.m.functions:
        for blk in f.blocks:
            new_list = []
            for inst in blk.instructions:
                si = getattr(inst, "sync_info", None)
                if si is not None and si.on_wait and len(si.on_wait) > max_waits:
                    waits = list(si.on_wait)
                    extra, keep = waits[:-max_waits], waits[-max_waits:]
                    for wcond in extra:
                        ctr[0] += 1
                        nop = mybir.InstDrain(
                            name=f"waitsplit_{ctr[0]}", ins=[], outs=[]
                        )
                        nop.engine = inst.engine
                        nop.sync_info = mybir.SyncInfo(on_wait=[wcond], on_update=[])
                        new_list.append(nop)
                        nc.register_instruction(nop, overwrite=True)
                    si.on_wait = keep
                new_list.append(inst)
            blk.instructions = new_list
    return nc


def _emit_pass(nc, src, wt, dst, p):
    """One LGA pass: src slab -> dst (slab for p<3, compact output for p=3)."""
    YB = {1: 28, 2: 26, 3: 24}[p]
    L0s = {1: (2, 30), 2: (4, 30), 3: (6, 30)}[p]
    with TileContext(nc) as tc:
        with (
            tc.tile_pool(name=f"cost{p}", bufs=2) as cpool,
            tc.tile_pool(name=f"w{p}", bufs=2) as wpool,
            tc.tile_pool(name=f"acc{p}", bufs=2) as apool,
            tc.tile_pool(name=f"q{p}", bufs=2) as qpool,
        ):
            for b in range(2):
                for xc in range(6):
                    for L0 in L0s:
                        rows = YB + 4
                        ct = cpool.tile([128, rows, 330], F32)
                        nc.sync.dma_start(
                            out=ct[:],
                            in_=bass.AP(
                                tensor=src,
                                offset=b * S_B + (L0 - 2) * S_ROW + xc * 128 * S_X,
                                ap=[[S_X, 128], [S_ROW, rows], [1, 330]],
                            ),
                        )
                        wtile = wpool.tile([128, YB, 75], F32)
                        nc.sync.dma_start(
                            out=wtile[:],
                            in_=bass.AP(
                                tensor=wt,
                                offset=b * W_B + (L0 - 2) * W_ROW + xc * 128 * W_X,
                                ap=[[W_X, 128], [W_ROW, YB], [1, 75]],
                            ),
                        )
                        acc = apool.tile([128, YB, 64], F32)
                        q = qpool.tile([128, YB, 64], F32)
                        for t in range(75):
                            g, ij = t // 25, t % 25
                            i, jj = ij // 5, ij % 5
                            base = 66 * jj + g
                            src_ap = ct[:, i : i + YB, base : base + 64]
                            w_ap = wtile[:, 0:YB, t : t + 1].broadcast_to(
                                [128, YB, 64]
                            )
                            if t == 0:
                                nc.vector.tensor_mul(out=acc[:], in0=src_ap, in1=w_ap)
                            else:
                                nc.vector.tensor_mul(out=q[:], in0=src_ap, in1=w_ap)
                                nc.vector.tensor_add(out=acc[:], in0=acc[:], in1=q[:])
                        if p < 3:
                            dst_ap = bass.AP(
                                tensor=dst,
                                offset=b * S_B
                                + L0 * S_ROW
                                + (xc * 128 + 2) * S_X
                                + 1,
                                ap=[[S_X, 128], [S_ROW, YB], [1, 64]],
                            )
                        else:
                            dst_ap = bass.AP(
                                tensor=dst,
                                offset=b * O_B + (L0 - 6) * O_ROW + xc * 128 * O_X,
                                ap=[[O_X, 128], [O_ROW, YB], [1, 64]],
                            )
                        nc.sync.dma_start(out=dst_ap, in_=acc[:])


def _build():
    nc = bass.Bass()
    a = nc.dram_tensor("a", SLAB_SHAPE, F32, kind="ExternalInput")
    w = nc.dram_tensor("w", W_SHAPE, F32, kind="ExternalInput")
    bs = nc.dram_tensor("bslab", SLAB_SHAPE, F32, kind="Internal")
    cs = nc.dram_tensor("cslab", SLAB_SHAPE, F32, kind="Internal")
    o = nc.dram_tensor("o", O_SHAPE, F32, kind="ExternalOutput")

    # ctx 0: zero both intermediate slabs (borders must read as zero pad)
    total = 2 * 60 * 772 * 66
    with TileContext(nc) as tc:
        with tc.tile_pool(name="z", bufs=1) as zp:
            zt = zp.tile([128, 1024], F32)
            nc.vector.memset(zt[:], 0.0)
            for dstt in (bs, cs):
                off = 0
                while off < total:
                    n = min(128 * 1024, total - off)
                    cols = n // 128
                    nc.sync.dma_start(
                        out=bass.AP(
                            tensor=dstt, offset=off, ap=[[cols, 128], [1, cols]]
                        ),
                        in_=zt[:, :cols],
                    )
                    off += n

    _emit_pass(nc, a, w, bs, 1)
    _emit_pass(nc, bs, w, cs, 2)
    _emit_pass(nc, cs, w, o, 3)
    _split_waits(nc)
    return nc


_NC_CACHE = [None]


def _prepare(input1: np.ndarray, input2: np.ndarray):
    input1 = np.asarray(input1, dtype=np.float32)
    input2 = np.asarray(input2, dtype=np.float32)
    if _NC_CACHE[0] is None:
        _NC_CACHE[0] = _build()
    nc = _NC_CACHE[0]

    in_maps = []
    for k in range(N_CORES):
        s = k * ROWS
        slab = np.zeros((2, 60, 772, 66), np.float32)
        lo, hi = max(0, s - 6), min(H, s + 54)
        slab[:, lo - (s - 6) : hi - (s - 6), 2:770, 1:65] = input1[
            :, :, lo:hi, :
        ].transpose(0, 2, 3, 1)
        wsl = np.zeros((2, 56, 768, 75), np.float32)
        lo2, hi2 = max(0, s - 4), min(H, s + 52)
        wsl[:, lo2 - (s - 4) : hi2 - (s - 4), :, :] = input2[
            :, :, lo2:hi2, :
        ].transpose(0, 2, 3, 1)
        in_maps.append({"a": slab, "w": wsl})
    return nc, in_maps


def kernel(input1: np.ndarray, input2: np.ndarray) -> np.ndarray:
    nc, in_maps = _prepare(input1, input2)

    trace = os.environ.get("LGA3_TRACE", "") not in ("", "0")
    res = bass_utils.run_bass_kernel_spmd(
        nc, in_maps, core_ids=list(range(N_CORES)), trace=trace
    )
    LAST_EXEC_NS[0] = res.exec_time_ns

    out = np.empty((B, D, H, W), np.float32)
    for k in range(N_CORES):
        s = k * ROWS
        out[:, :, s : s + ROWS, :] = res.results[k]["o"].transpose(0, 3, 1, 2)
    return out



# revision 3
# speedup vs baseline: 1.1447x; 1.1447x over previous
"""LGA3 (3x local guided aggregation, radius 2) on 8 TRN2 NeuronCores.

bf16 DVE-2x version. Same sharding as the fp32 baseline: H split 8 ways
(48 output rows/core), redundant-halo compute (pass1 56 rows, pass2 52,
pass3 48) -> no inter-core comms.

Key speedup: every tensor_mul/tensor_add runs in the DVE 2x_1P packed
mode (2 elem/cycle) instead of fp32 1x. Requirements: all operands bf16
with innermost AP dim step +-1 (>=2 elems) and 4B-aligned starts.
 - The weight broadcast along d (step 0) breaks packing, so weights are
   stored host-side as duplicated pairs [w,w] (150 per pixel); the mul
   reads cost as d-pairs [2,32-steps] and weights as a [0,32]x[1,2] AP.
 - Pair reads need even element offsets. The three disparity groups read
   at d-offsets {0,1,2}, so two cost slabs are kept: slab A with d pad 1
   (groups 0/2 at even offsets 66*jj+{0,2}) and slab B unpadded
   (group 1 at 64*jj).
 - bf16 accumulation error is controlled with 4 interleaved accumulator
   chains (~19 taps each) combined in fp32 at the end of each pixel
   block; the final combine writes the next pass's slabs (bf16) or the
   fp32 output.
"""

import os
import sys

for _p in ("/opt/trn_rl_repo", "/root/.axon_site/_ro/trn_rl_repo"):
    if os.path.isdir(_p) and _p not in sys.path:
        sys.path.append(_p)

import numpy as np
import ml_dtypes
import concourse.bass as bass
import concourse.mybir as mybir
from concourse.tile import TileContext
from concourse import bass_utils

F32 = mybir.dt.float32
BF16 = mybir.dt.bfloat16
BF = ml_dtypes.bfloat16

B, D, H, W = 2, 64, 384, 768
N_CORES = 8
ROWS = H // N_CORES  # 48 output rows per core

# slab A [b, 60 rows, 772 x, 66 d]: plane k = cost d=k-1 (strides in elements)
SA_X, SA_ROW, SA_B = 66, 772 * 66, 60 * 772 * 66
A_SHAPE = [2, 60, 772, 66]
# slab B [b, 60 rows, 772 x, 64 d]: plane k = cost d=k
SB_X, SB_ROW, SB_B = 64, 772 * 64, 60 * 772 * 64
B_SHAPE = [2, 60, 772, 64]
# weight slab [b, 56 rows, 768 x, 150 t] (each of 75 weights duplicated)
W_X, W_ROW, W_B = 150, 768 * 150, 56 * 768 * 150
W_SHAPE = [2, 56, 768, 150]
# output [b, 48 rows, 768 x, 64 d] fp32
O_X, O_ROW, O_B = 64, 768 * 64, 48 * 768 * 64
O_SHAPE = [2, 48, 768, 64]

NCHAIN = 4

LAST_EXEC_NS = [None]
LAST_RES = [None]


def _split_waits(nc, max_waits=1):
    """Split >max_waits sync waits on one instruction into preceding
    wait-only drains (walrus setupSyncWait limit workaround)."""
    ctr = [0]
    for f in nc.m.functions:
        for blk in f.blocks:
            new_list = []
            for inst in blk.instructions:
                si = getattr(inst, "sync_info", None)
                if si is not None and si.on_wait and len(si.on_wait) > max_waits:
                    waits = list(si.on_wait)
                    extra, keep = waits[:-max_waits], waits[-max_waits:]
                    for wcond in extra:
                        ctr[0] += 1
                        nop = mybir.InstDrain(
                            name=f"waitsplit_{ctr[0]}", ins=[], outs=[]
                        )
                        nop.engine = inst.engine
                        nop.sync_info = mybir.SyncInfo(on_wait=[wcond], on_update=[])
                        new_list.append(nop)
                        nc.register_instruction(nop, overwrite=True)
                    si.on_wait = keep
                new_list.append(inst)
            blk.instructions = new_list
    return nc


def _pair(ap):
    """[p, r, 64] -> [p, r, 32, 2] paired view (innermost step-1 pairs)."""
    return ap.rearrange("p r (d two) -> p r d two", two=2)


def _emit_pass(nc, srcA, srcB, wt, dstA, dstB, dstO, p):
    """One LGA pass reading slab pair (srcA, srcB); writes slab pair
    (dstA, dstB) for p<3 or the compact fp32 output dstO for p=3."""
    YB = {1: 28, 2: 26, 3: 24}[p]
    L0s = {1: (2, 30), 2: (4, 30), 3: (6, 30)}[p]
    with TileContext(nc) as tc:
        with (
            tc.tile_pool(name=f"ca{p}", bufs=2) as capool,
            tc.tile_pool(name=f"cb{p}", bufs=2) as cbpool,
            tc.tile_pool(name=f"w{p}", bufs=2) as wpool,
            tc.tile_pool(name=f"acc{p}", bufs=2) as apool,
            tc.tile_pool(name=f"q{p}", bufs=2) as qpool,
            tc.tile_pool(name=f"comb{p}", bufs=2) as combpool,
            tc.tile_pool(name=f"fin{p}", bufs=2) as finpool,
        ):
            for b in range(2):
                for xc in range(6):
                    for L0 in L0s:
                        rows = YB + 4
                        at = capool.tile([128, rows, 330], BF16, name=f"at{p}")
                        nc.sync.dma_start(
                            out=at[:],
                            in_=bass.AP(
                                tensor=srcA,
                                offset=b * SA_B + (L0 - 2) * SA_ROW + xc * 128 * SA_X,
                                ap=[[SA_X, 128], [SA_ROW, rows], [1, 330]],
                            ),
                        )
                        bt = cbpool.tile([128, rows, 320], BF16, name=f"bt{p}")
                        nc.sync.dma_start(
                            out=bt[:],
                            in_=bass.AP(
                                tensor=srcB,
                                offset=b * SB_B + (L0 - 2) * SB_ROW + xc * 128 * SB_X,
                                ap=[[SB_X, 128], [SB_ROW, rows], [1, 320]],
                            ),
                        )
                        wtile = wpool.tile([128, YB, 150], BF16, name=f"wt{p}")
                        nc.sync.dma_start(
                            out=wtile[:],
                            in_=bass.AP(
                                tensor=wt,
                                offset=b * W_B + (L0 - 2) * W_ROW + xc * 128 * W_X,
                                ap=[[W_X, 128], [W_ROW, YB], [1, 150]],
                            ),
                        )
                        accs = [
                            apool.tile([128, YB, 64], BF16, name=f"acc{c}_{p}")
                            for c in range(NCHAIN)
                        ]
                        acc4 = [_pair(a[:]) for a in accs]
                        q = qpool.tile([128, YB, 64], BF16, name=f"q{p}")
                        q4 = _pair(q[:])
                        started = [False] * NCHAIN
                        for t in range(75):
                            g, ij = t // 25, t % 25
                            i, jj = ij // 5, ij % 5
                            if g == 1:
                                base = 64 * jj
                                src4 = _pair(bt[:, i : i + YB, base : base + 64])
                            else:
                                base = 66 * jj + g
                                src4 = _pair(at[:, i : i + YB, base : base + 64])
                            w4 = (
                                wtile[:, 0:YB, 2 * t : 2 * t + 2]
                                .unsqueeze(2)
                                .broadcast_to([128, YB, 32, 2])
                            )
                            c = t % NCHAIN
                            if not started[c]:
                                nc.vector.tensor_mul(out=acc4[c], in0=src4, in1=w4)
                                started[c] = True
                            else:
                                nc.vector.tensor_mul(out=q4, in0=src4, in1=w4)
                                nc.vector.tensor_add(
                                    out=acc4[c], in0=acc4[c], in1=q4
                                )
                        c01 = combpool.tile([128, YB, 64], F32, name=f"c01_{p}")
                        c23 = combpool.tile([128, YB, 64], F32, name=f"c23_{p}")
                        nc.vector.tensor_add(out=c01[:], in0=accs[0][:], in1=accs[1][:])
                        nc.vector.tensor_add(out=c23[:], in0=accs[2][:], in1=accs[3][:])
                        if p < 3:
                            fin = finpool.tile([128, YB, 64], BF16, name=f"fin{p}")
                            nc.vector.tensor_add(out=fin[:], in0=c01[:], in1=c23[:])
                            nc.sync.dma_start(
                                out=bass.AP(
                                    tensor=dstA,
                                    offset=b * SA_B
                                    + L0 * SA_ROW
                                    + (xc * 128 + 2) * SA_X
                                    + 1,
                                    ap=[[SA_X, 128], [SA_ROW, YB], [1, 64]],
                                ),
                                in_=fin[:],
                            )
                            nc.sync.dma_start(
                                out=bass.AP(
                                    tensor=dstB,
                                    offset=b * SB_B
                                    + L0 * SB_ROW
                                    + (xc * 128 + 2) * SB_X,
                                    ap=[[SB_X, 128], [SB_ROW, YB], [1, 64]],
                                ),
                                in_=fin[:],
                            )
                        else:
                            fin = finpool.tile([128, YB, 64], F32, name=f"fin{p}")
                            nc.vector.tensor_add(out=fin[:], in0=c01[:], in1=c23[:])
                            nc.sync.dma_start(
                                out=bass.AP(
                                    tensor=dstO,
                                    offset=b * O_B + (L0 - 6) * O_ROW + xc * 128 * O_X,
                                    ap=[[O_X, 128], [O_ROW, YB], [1, 64]],
                                ),
                                in_=fin[:],
                            )


def _build():
    nc = bass.Bass()
    a = nc.dram_tensor("a", A_SHAPE, BF16, kind="ExternalInput")
    bsl = nc.dram_tensor("bsl", B_SHAPE, BF16, kind="ExternalInput")
    w = nc.dram_tensor("w", W_SHAPE, BF16, kind="ExternalInput")
    pA = nc.dram_tensor("pa", A_SHAPE, BF16, kind="Internal")
    pB = nc.dram_tensor("pb", B_SHAPE, BF16, kind="Internal")
    qA = nc.dram_tensor("qa", A_SHAPE, BF16, kind="Internal")
    qB = nc.dram_tensor("qb", B_SHAPE, BF16, kind="Internal")
    o = nc.dram_tensor("o", O_SHAPE, F32, kind="ExternalOutput")

    # ctx 0: zero the intermediate slabs (borders must read as zero pad)
    with TileContext(nc) as tc:
        with tc.tile_pool(name="z", bufs=1) as zp:
            zt = zp.tile([128, 1024], BF16)
            nc.vector.memset(zt[:], 0.0)
            for dstt, total in (
                (pA, 2 * 60 * 772 * 66),
                (qA, 2 * 60 * 772 * 66),
                (pB, 2 * 60 * 772 * 64),
                (qB, 2 * 60 * 772 * 64),
            ):
                off = 0
                while off < total:
                    n = min(128 * 1024, total - off)
                    cols = n // 128
                    nc.sync.dma_start(
                        out=bass.AP(
                            tensor=dstt, offset=off, ap=[[cols, 128], [1, cols]]
                        ),
                        in_=zt[:, :cols],
                    )
                    off += n

    _emit_pass(nc, a, bsl, w, pA, pB, None, 1)
    _emit_pass(nc, pA, pB, w, qA, qB, None, 2)
    _emit_pass(nc, qA, qB, w, None, None, o, 3)
    _split_waits(nc)
    return nc


_NC_CACHE = [None]


def _prepare(input1: np.ndarray, input2: np.ndarray):
    """Build (or reuse) the bass program and the per-core input maps."""
    input1 = np.asarray(input1, dtype=np.float32)
    input2 = np.asarray(input2, dtype=np.float32)
    if _NC_CACHE[0] is None:
        _NC_CACHE[0] = _build()
    nc = _NC_CACHE[0]

    t1 = input1.transpose(0, 2, 3, 1).astype(BF)  # [2, 384, 768, 64]
    fullA = np.zeros((2, H + 12, 772, 66), BF)
    fullA[:, 6 : 6 + H, 2:770, 1:65] = t1
    fullB = np.zeros((2, H + 12, 772, 64), BF)
    fullB[:, 6 : 6 + H, 2:770, :] = t1
    wT = input2.transpose(0, 2, 3, 1).astype(BF)  # [2, 384, 768, 75]
    fullW = np.zeros((2, H + 8, 768, 150), BF)
    fullW[:, 4 : 4 + H, :, 0::2] = wT
    fullW[:, 4 : 4 + H, :, 1::2] = wT

    in_maps = []
    for k in range(N_CORES):
        s = k * ROWS
        in_maps.append(
            {
                "a": np.ascontiguousarray(fullA[:, s : s + 60]),
                "bsl": np.ascontiguousarray(fullB[:, s : s + 60]),
                "w": np.ascontiguousarray(fullW[:, s : s + 56]),
            }
        )
    return nc, in_maps


def kernel(input1: np.ndarray, input2: np.ndarray) -> np.ndarray:
    nc, in_maps = _prepare(input1, input2)

    trace = os.environ.get("LGA3_TRACE", "") not in ("", "0")
    res = bass_utils.run_bass_kernel_spmd(
        nc, in_maps, core_ids=list(range(N_CORES)), trace=trace
    )
    LAST_EXEC_NS[0] = res.exec_time_ns
    LAST_RES[0] = res

    out = np.empty((B, D, H, W), np.float32)
    for k in range(N_CORES):
        s = k * ROWS
        out[:, :, s : s + ROWS, :] = res.results[k]["o"].transpose(0, 3, 1, 2)
    return out
